# revision 1
# baseline (speedup 1.0000x reference)
"""Trainium2 Bass kernel for nn_AttentionBlock_73323681677485.

out = x + BN(softmax_k(sum_d scale_d * tanh(x_q + x_k)) @ x)

Algorithm: tanh(a+b) is replaced by a 10-term free-frequency sine series
    tanh(s) ~ sum_m c_m sin(w_m s),  |s| <= 9.8, max err 1.0e-4,
which is SEPARABLE: sin(w(a+b)) = sin(wa)cos(wb) + cos(wa)sin(wb). The
[Tq, Tk] score block therefore becomes M accumulating matmuls over a
2*M*D-long feature axis on the TensorEngine instead of 268M scalar tanh
evaluations.

Per-core pipeline (SPMD, 8 cores = 4 batches x 2 query halves):
  gpsimd:  y = x*w_m/2pi + phase          (phase 0 / 0.25 stacks sin|cos
                                           on the 2x64 duplicated-d axis)
  vector:  n = rint(y)  (fp32 magic-constant trick),  r = y - n
  scalar:  F = sin(2pi r);  QF = F_q * (c_m scale_d)  (Copy w/ AP scale)
  tensor:  scoresT[k,q] += F_k_chunk^T @ QF   (fp32r, 8 PSUM banks)
  scalar:  e = exp(scoresT)      (no max-pass: |scores| <= 32, exp-safe;
                                  table set prefetched)
  tensor:  ctx1[q, 0:66] += e_chunk^T @ [x*A | 1 | 0]   (BN scale A and
           the softmax denominator folded into the key matrix columns)
  vector:  out = ctx1[:, :64] * (1/ctx1[:, 64]) + (x_q + BN shift)

The harness-facing entry point is kernel(**inputs) -> np.ndarray.
"""
import numpy as np

# ---- fitted sine series for tanh, density-weighted fit on |s|<=9.8 ----
FOURIER_W = [
    0.2705667961754151,
    1.3709545010797533,
    0.815676176160012,
    2.5210935575532263,
    1.9392495765936073,
    3.1161283956725363,
    3.722344952044567,
    4.337282997833672,
    5.211242198935298,
]
FOURIER_C = [
    1.2372973584284093,
    0.13215038316519057,
    0.3306436725515483,
    0.022440765066042537,
    0.05480553663256792,
    0.009012416759483449,
    0.003493521722665886,
    0.0014695578359931781,
    0.0006565769512427879,
]
FOURIER_M = len(FOURIER_W)

MAGIC = np.float32(12582912.0)   # 1.5*2^23; y+MAGIC-MAGIC == rint(y)
TWO_PI = float(2.0 * np.pi)

B, T, D = 4, 1024, 64
NCORES = 8
QPC = (B * T) // NCORES          # queries per core = 512
KT = T // 128                    # key tiles = 8
QT = QPC // 128                  # query tiles per core = 4
BN_EPS = 1e-3                    # Keras BatchNormalization default

_nc_cache = {}


# --------------------------------------------------------------------------
# TileContext variant: this container's walrus accepts at most ONE sync wait
# per instruction; hoist extra waits onto same-engine NoOps and split the
# kernel-tail drain into single-wait SP nops.
# --------------------------------------------------------------------------
def _make_tile_context_cls():
    import re
    import bass_rust
    import concourse.mybir as mybir
    from concourse.tile import TileContext, ScopedClock

    def _clock_ticks(vc):
        m = re.search(r"VectorClock\(\[([0-9, ]*)\]\)", repr(vc))
        return ([int(s) for s in m.group(1).split(",")]
                if m.group(1).strip() else [])

    class SplitWaitTileContext(TileContext):
        _ws_counter = 0

        def _commit_instruction(self, inst, lazy_reg_writes=True):
            si = inst.sync_info
            if (si is not None and si.on_wait and len(si.on_wait) > 1
                    and inst.engine != mybir.EngineType.Unassigned):
                waits = list(si.on_wait)
                for w in waits[:-1]:
                    SplitWaitTileContext._ws_counter += 1
                    nop = mybir.InstNoOp(
                        name=f"{inst.name}-ws{SplitWaitTileContext._ws_counter}",
                        ins=[], outs=[])
                    nop.engine = inst.engine
                    nop.sync_info = mybir.SyncInfo(on_wait=[w], on_update=[])
                    super()._commit_instruction(nop, lazy_reg_writes=False)
                inst.sync_info = mybir.SyncInfo(
                    on_wait=[waits[-1]], on_update=list(si.on_update or []))
            return super()._commit_instruction(inst, lazy_reg_writes)

        def _drain_and_barrier(self, tick_clock, wait_clock):
            ticks = _clock_ticks(tick_clock.global_clock)
            n = len(ticks)
            for i, t in enumerate(ticks):
                if t > 0:
                    v = [0] * n
                    v[i] = t
                    nop = self.nc.sync.nop(nofuse=True)
                    wait_clock.add_sem_waits(
                        nop.ins,
                        ScopedClock({None: bass_rust.VectorClock(v)}))
            self.nc.sync.drain()
            self.nc.all_engine_barrier()
            assert self.sems is not None
            popped = self.nc._tile_sem_poison_stack.pop()
            assert popped is self._sem_poison
            self.nc.clear_and_free_semaphores(
                list(self.sems.allocated().values()))

    return SplitWaitTileContext


def _big_layout(M):
    """Column layout of the consolidated fp32 input [128, NBIG]."""
    o = {}
    c = 0
    o["ph_k"] = (c, c + 1); c += 1
    o["ph_q"] = (c, c + 1); c += 1
    o["wv"] = (c, c + M); c += M
    o["x2k"] = (c, c + T); c += T
    o["x2q"] = (c, c + QPC); c += QPC
    o["xqc"] = (c, c + QT * D); c += QT * D
    return o, c


def build_nc(mm_dtype="float32r", rs_pool=0, feat_bufs=4):
    """Build the SPMD single-core Bass program (same program on 8 cores;
    all per-core variation lives in the input tensors)."""
    import concourse.bass as bass
    import concourse.mybir as mybir
    from contextlib import ExitStack

    TileCtx = _make_tile_context_cls()
    M = FOURIER_M
    ws = FOURIER_W
    f32 = mybir.dt.float32
    mdt = getattr(mybir.dt, mm_dtype)
    AF = mybir.ActivationFunctionType
    ALU = mybir.AluOpType
    lay, NBIG = _big_layout(M)

    nc = bass.Bass("TRN2", target_bir_lowering=False)
    big = nc.dram_tensor("big", [128, NBIG], f32, kind="ExternalInput")
    xk1 = nc.dram_tensor("xk1", [T, 66], f32, kind="ExternalInput")
    out = nc.dram_tensor("out", [QPC, D], f32, kind="ExternalOutput")

    with TileCtx(nc) as tc, ExitStack() as st:
        ins = st.enter_context(tc.tile_pool(name="ins", bufs=1))
        feat = st.enter_context(tc.tile_pool(name="feat", bufs=feat_bufs))
        epool = st.enter_context(tc.tile_pool(name="epool", bufs=1))
        small = st.enter_context(tc.tile_pool(name="small", bufs=4))

        big_t = ins.tile([128, NBIG], f32)
        split = lay["x2k"][1]
        nc.sync.dma_start(out=big_t[:, 0:split], in_=big[:, 0:split])
        nc.sync.dma_start(out=big_t[:, split:NBIG], in_=big[:, split:NBIG])
        xk1_t = ins.tile([128, KT, 66], f32)
        nc.sync.dma_start(out=xk1_t,
                          in_=xk1.rearrange("(c p) e -> p c e", p=128))

        def bs(name):
            a, b = lay[name]
            return big_t[:, a:b]
        ph_k_t, ph_q_t, wv_t = bs("ph_k"), bs("ph_q"), bs("wv")
        x2k_t, x2q_t = bs("x2k"), bs("x2q")
        xqc_t = bs("xqc").rearrange("p (j d) -> p j d", j=QT)

        # prefetch the trig table set while the first y/n/r chain runs
        dummy = small.tile([128, 1], f32, tag="dummy")
        nc.scalar.activation(out=dummy, in_=ph_k_t[:, 0:1], func=AF.Sin)

        # ------------- phase A: features + score matmuls -------------
        pscore_cm = tc.tile_pool(name="pscore", bufs=1, space="PSUM")
        pscore = pscore_cm.__enter__()
        sc_a = pscore.tile([128, 4, 512], mybir.dt.float32, name="scoresA")
        sc_b = pscore.tile([128, 4, 512], mybir.dt.float32, name="scoresB")

        def sc_slice(kt):
            return (sc_a if kt < 4 else sc_b)[:, kt % 4, :]
        W = T + QPC

        def emit_m(m, chunks):
            """One frequency's features + matmuls. chunks: ordered (a, b)
            column ranges over the [k | q] concatenated width; the q part
            (T..W) must be a single chunk and come first for fast QF."""
            w32 = float(np.float32(ws[m] / (2 * np.pi)))
            yall = feat.tile([128, W], f32, tag="yall")
            nall = feat.tile([128, W], f32, tag="nall")
            rall = feat.tile([128, W], f32, tag="rall")
            Fall = feat.tile([128, W], mdt, tag="Fall")
            QF = feat.tile([128, QPC], mdt, tag="QF")
            qf_done = False
            covered_k = 0
            mm_done = 0
            y_eng = nc.vector if m == 0 else nc.gpsimd
            for (a, b) in chunks:
                ph = ph_q_t if a >= T else ph_k_t
                src = x2q_t[:, a - T:b - T] if a >= T else x2k_t[:, a:b]
                y_eng.tensor_scalar(
                    out=yall[:, a:b], in0=src, scalar1=w32,
                    scalar2=ph[:, 0:1], op0=ALU.mult, op1=ALU.add)
                nc.vector.tensor_scalar(
                    out=nall[:, a:b], in0=yall[:, a:b],
                    scalar1=float(MAGIC), scalar2=-float(MAGIC),
                    op0=ALU.add, op1=ALU.add)
                rs = min(rs_pool, b - a) if a < T else 0
                nc.vector.tensor_tensor(
                    out=rall[:, a:b - rs], in0=yall[:, a:b - rs],
                    in1=nall[:, a:b - rs], op=ALU.subtract)
                if rs:
                    nc.gpsimd.tensor_tensor(
                        out=rall[:, b - rs:b], in0=yall[:, b - rs:b],
                        in1=nall[:, b - rs:b], op=ALU.subtract)
                nc.scalar.activation(out=Fall[:, a:b], in_=rall[:, a:b],
                                     func=AF.Sin, scale=TWO_PI)
                if a >= T:
                    nc.scalar.activation(
                        out=QF, in_=Fall[:, T:W], func=AF.Copy,
                        scale=wv_t[:, m:m + 1])
                    qf_done = True
                else:
                    covered_k = max(covered_k, b)
                while qf_done and (mm_done + 1) * 128 <= covered_k:
                    kt = mm_done
                    nc.tensor.matmul(
                        sc_slice(kt), Fall[:, kt * 128:(kt + 1) * 128],
                        QF, start=(m == 0), stop=(m == M - 1))
                    mm_done += 1
            assert mm_done == KT

        def emit_pair(m):
            """Two frequencies sharing one wide sin activation."""
            y2 = feat.tile([128, 2, W], f32, tag="yall")
            n2 = feat.tile([128, 2, W], f32, tag="nall")
            r2 = feat.tile([128, 2, W], f32, tag="rall")
            F2 = feat.tile([128, 2, W], mdt, tag="Fall")
            for i in (0, 1):
                w32 = float(np.float32(ws[m + i] / (2 * np.pi)))
                nc.gpsimd.tensor_scalar(
                    out=y2[:, i, 0:T], in0=x2k_t, scalar1=w32,
                    scalar2=ph_k_t[:, 0:1], op0=ALU.mult, op1=ALU.add)
                nc.gpsimd.tensor_scalar(
                    out=y2[:, i, T:W], in0=x2q_t, scalar1=w32,
                    scalar2=ph_q_t[:, 0:1], op0=ALU.mult, op1=ALU.add)
                nc.vector.tensor_scalar(
                    out=n2[:, i, :], in0=y2[:, i, :], scalar1=float(MAGIC),
                    scalar2=-float(MAGIC), op0=ALU.add, op1=ALU.add)
                nc.vector.tensor_tensor(out=r2[:, i, :], in0=y2[:, i, :],
                                        in1=n2[:, i, :], op=ALU.subtract)
            nc.scalar.activation(out=F2, in_=r2, func=AF.Sin, scale=TWO_PI)
            for i in (0, 1):
                QF = feat.tile([128, QPC], mdt, tag="QF")
                nc.scalar.activation(
                    out=QF, in_=F2[:, i, T:W], func=AF.Copy,
                    scale=wv_t[:, m + i:m + i + 1])
                for kt in range(KT):
                    nc.tensor.matmul(
                        sc_slice(kt), F2[:, i, kt * 128:(kt + 1) * 128],
                        QF, start=(m + i == 0), stop=(m + i == M - 1))

        warm = [(T, W), (0, 512), (512, T)]
        full = [(T, W), (0, T)]
        for m in range(M):
            emit_m(m, warm if m < 2 else full)

        # ------------- phase B+C: exp split (kt-half x qtile) --------
        # separate tiles per (half, qtile) give the finest dependency
        # granularity: ctx for qtile j starts right after its two exps.
        e_a = epool.tile([128, 4, 512], f32, name="e_a")
        nc.scalar.activation(out=e_a, in_=sc_a, func=AF.Exp)
        e_b = epool.tile([128, 4, 512], f32, name="e_b")
        nc.scalar.activation(out=e_b, in_=sc_b, func=AF.Exp)
        pscore_cm.__exit__(None, None, None)
        pctx = st.enter_context(
            tc.tile_pool(name="pctx", bufs=2, space="PSUM"))
        out4 = epool.tile([128, QT, D], f32, name="out4")
        for j in range(QT):
            cps = pctx.tile([128, 66], mybir.dt.float32, tag="cps")
            for kt in range(KT):
                eh = e_a if kt < 4 else e_b
                nc.tensor.matmul(
                    cps, eh[:, kt % 4, j * 128:(j + 1) * 128],
                    xk1_t[:, kt, :], start=(kt == 0), stop=(kt == KT - 1))
            invs = small.tile([128, 1], f32, tag="invs")
            nc.vector.reciprocal(out=invs, in_=cps[:, 64:65])
            t1 = small.tile([128, D], f32, tag="t1")
            nc.vector.tensor_scalar_mul(out=t1, in0=cps[:, 0:64],
                                        scalar1=invs[:, 0:1])
            nc.gpsimd.tensor_tensor(out=out4[:, j, :], in0=t1,
                                    in1=xqc_t[:, j, :], op=ALU.add)
            nc.sync.dma_start(
                out=out.rearrange("(j p) d -> p j d", p=128)[:, j, :],
                in_=out4[:, j, :])
    return nc


def host_prep(x, scale, gamma, beta, moving_mean, moving_var):
    """Per-core input maps. BN scale A is folded into the key-matrix
    columns, BN shift C into the residual term; a ones column appended to
    the keys yields the softmax denominator from the same matmul."""
    M = FOURIER_M
    cs = FOURIER_C
    lay, NBIG = _big_layout(M)
    x = np.ascontiguousarray(x, dtype=np.float32)
    A = (np.asarray(gamma, np.float64)
         / np.sqrt(np.asarray(moving_var, np.float64) + BN_EPS)
         ).astype(np.float32)
    Cc = (np.asarray(beta, np.float32) - np.asarray(moving_mean, np.float32)
          * A).astype(np.float32)
    scale = np.asarray(scale, np.float32)

    wvec = np.empty((128, M), np.float32)
    for m in range(M):
        wvec[:64, m] = cs[m] * scale
        wvec[64:, m] = cs[m] * scale
    ph_k = np.zeros((128, 1), np.float32); ph_k[64:] = 0.25   # [sin; cos]
    ph_q = np.zeros((128, 1), np.float32); ph_q[:64] = 0.25   # [cos; sin]

    in_maps = []
    for c in range(NCORES):
        b, h = divmod(c, 2)
        q0 = h * QPC
        xb = x[b]                                    # [T, D]
        xt = np.ascontiguousarray(xb.T)              # [D, T]
        x2k = np.concatenate([xt, xt], 0)            # [128, T]
        xk1 = np.concatenate(
            [xb * A[None, :], np.ones((T, 1), np.float32),
             np.zeros((T, 1), np.float32)], 1)       # [T, 66]
        xqc = (xb[q0:q0 + QPC] + Cc).astype(np.float32)
        xqc_pjd = np.transpose(
            xqc.reshape(QT, 128, D), (1, 0, 2)).reshape(128, QT * D)
        bigv = np.empty((128, NBIG), np.float32)
        bigv[:, lay["ph_k"][0]:lay["ph_k"][1]] = ph_k
        bigv[:, lay["ph_q"][0]:lay["ph_q"][1]] = ph_q
        bigv[:, lay["wv"][0]:lay["wv"][1]] = wvec
        bigv[:, lay["x2k"][0]:lay["x2k"][1]] = x2k
        bigv[:, lay["x2q"][0]:lay["x2q"][1]] = x2k[:, q0:q0 + QPC]
        bigv[:, lay["xqc"][0]:lay["xqc"][1]] = xqc_pjd
        in_maps.append({
            "big": bigv,
            "xk1": np.ascontiguousarray(xk1.astype(np.float32)),
        })
    return in_maps


def kernel(x, scale, gamma, beta, moving_mean, moving_var):
    import sys
    from concourse.bass_utils import run_bass_kernel_spmd
    key = "float32r"
    if key not in _nc_cache:
        _nc_cache[key] = build_nc(mm_dtype=key)
    nc = _nc_cache[key]
    in_maps = host_prep(x, scale, gamma, beta, moving_mean, moving_var)
    res = run_bass_kernel_spmd(nc, in_maps, core_ids=list(range(NCORES)))
    out = np.empty((B, T, D), np.float32)
    for c in range(NCORES):
        b, h = divmod(c, 2)
        q0 = h * QPC
        out[b, q0:q0 + QPC] = res.results[c]["out"]
    return out



# revision 15
# speedup vs baseline: 1.3050x; 1.3050x over previous
"""Trainium2 Bass kernel for nn_AttentionBlock_73323681677485.

out = x + BN(softmax_k(sum_d scale_d * tanh(x_q + x_k)) @ x)

Algorithm: tanh(s) ~ alpha*s + sum_m c_m sin(w_m s) (6 free-frequency
terms, density-weighted fit on |s|<=9.8). The sine part is SEPARABLE:
sin(w(a+b)) = sin(wa)cos(wb) + cos(wa)sin(wb), so the [Tq, Tk] score
block becomes 6 accumulating fp32r matmuls per key tile. The linear
term costs nothing on device: alpha*u_q is constant per softmax row
(cancels), alpha*u_k is folded on the host into the key-value matrix
as a per-key factor g_k = exp(alpha * x_k . scale).

Range reduction for sin uses the fp32 magic-constant rint trick:
    y = (w/2pi) x + ph,  n = (y + 1.5*2^23) - 1.5*2^23 = rint(y),
    r = y - n,  sin(2pi r) = sin(w x + 2pi ph).

Per-core pipeline (SPMD, 8 cores = 4 batches x 2 query halves):
  pool:   y = x*w/2pi + ph             (q cols use ph_q, k cols ph_k)
  vector: n = rint(y) (magic add);  r = y - n
  scalar: F = sin(2pi r);  QF = F_q * (c_m scale_d)
  tensor: scoresT[k,q] += F_k_chunk^T @ QF   (fp32r, 8 PSUM banks;
          dummy warm-up matmuls hold the PE p-state at full clock)
  scalar: e = exp(scoresT) -> bf16     (|scores| <= ~21, exp-safe)
  tensor: ctx[q, 0:66] += e_chunk^T @ [x*A*g | g | 0]  (bf16, BN scale
          A, softmax denominator and linear-term g folded into keys)
  vector: out = ctx[:, :64] * (1/ctx[:, 64]) + (x_q + BN shift)

The harness-facing entry point is kernel(**inputs) -> np.ndarray.
"""
import numpy as np

# ---- tanh(s) ~ ALPHA*s + sum c_m sin(w_m s), |s|<=9.8 density fit ----
ALPHA = 0.17708028376063317
FOURIER_W = [
    0.5580190998921836,
    1.12424452740956,
    1.706488423099657,
    2.2917784468632156,
    2.9779315459311255,
    4.023143816154509,
]
FOURIER_C = [
    0.5650001852330417,
    0.20179388641743914,
    0.08117239291144143,
    0.032323974205794956,
    0.015519041889326173,
    0.004560899561457579,
]
FOURIER_M = len(FOURIER_W)

TWO_PI = float(2.0 * np.pi)
MAGIC = np.float32(12582912.0)   # 1.5*2^23; y+MAGIC-MAGIC == rint(y)

B, T, D = 4, 1024, 64
NCORES = 8
QPC = (B * T) // NCORES          # queries per core = 512
KT = T // 128                    # key tiles = 8
QT = QPC // 128                  # query tiles per core = 4
BN_EPS = 1e-3                    # Keras BatchNormalization default
N_DUMMY_MM = 10                  # PE p-state warm-up matmuls

_nc_cache = {}


# --------------------------------------------------------------------------
# TileContext variant: this container's walrus accepts at most ONE sync wait
# per instruction; hoist extra waits onto same-engine NoOps and split the
# kernel-tail drain into single-wait SP nops.
# --------------------------------------------------------------------------
def _make_tile_context_cls():
    import re
    import bass_rust
    import concourse.mybir as mybir
    from concourse.tile import TileContext, ScopedClock

    def _clock_ticks(vc):
        m = re.search(r"VectorClock\(\[([0-9, ]*)\]\)", repr(vc))
        return ([int(s) for s in m.group(1).split(",")]
                if m.group(1).strip() else [])

    class SplitWaitTileContext(TileContext):
        _ws_counter = 0

        def _commit_instruction(self, inst, lazy_reg_writes=True):
            si = inst.sync_info
            if (si is not None and si.on_wait and len(si.on_wait) > 1
                    and inst.engine != mybir.EngineType.Unassigned):
                waits = list(si.on_wait)
                for w in waits[:-1]:
                    SplitWaitTileContext._ws_counter += 1
                    nop = mybir.InstNoOp(
                        name=f"{inst.name}-ws{SplitWaitTileContext._ws_counter}",
                        ins=[], outs=[])
                    nop.engine = inst.engine
                    nop.sync_info = mybir.SyncInfo(on_wait=[w], on_update=[])
                    super()._commit_instruction(nop, lazy_reg_writes=False)
                inst.sync_info = mybir.SyncInfo(
                    on_wait=[waits[-1]], on_update=list(si.on_update or []))
            return super()._commit_instruction(inst, lazy_reg_writes)

        def _drain_and_barrier(self, tick_clock, wait_clock):
            ticks = _clock_ticks(tick_clock.global_clock)
            n = len(ticks)
            for i, t in enumerate(ticks):
                if t > 0:
                    v = [0] * n
                    v[i] = t
                    nop = self.nc.sync.nop(nofuse=True)
                    wait_clock.add_sem_waits(
                        nop.ins,
                        ScopedClock({None: bass_rust.VectorClock(v)}))
            self.nc.sync.drain()
            self.nc.all_engine_barrier()
            assert self.sems is not None
            popped = self.nc._tile_sem_poison_stack.pop()
            assert popped is self._sem_poison
            self.nc.clear_and_free_semaphores(
                list(self.sems.allocated().values()))

    return SplitWaitTileContext


def _big_layout(M):
    """Column layout of the consolidated fp32 input [128, NBIG].
    x2q before x2k so the first (smallest) DMA unblocks the QF chain."""
    o = {}
    c = 0
    o["ph_k"] = (c, c + 1); c += 1
    o["ph_q"] = (c, c + 1); c += 1
    o["wv"] = (c, c + M); c += M
    o["x2q"] = (c, c + QPC); c += QPC
    o["x2k"] = (c, c + T); c += T
    o["xqc"] = (c, c + QT * D); c += QT * D
    return o, c


def build_nc(mm_dtype="float32r"):
    """Build the SPMD single-core Bass program (same program on 8 cores;
    all per-core variation lives in the input tensors)."""
    import concourse.bass as bass
    import concourse.mybir as mybir
    from contextlib import ExitStack

    TileCtx = _make_tile_context_cls()
    M = FOURIER_M
    ws = FOURIER_W
    f32 = mybir.dt.float32
    bf16 = mybir.dt.bfloat16
    mdt = getattr(mybir.dt, mm_dtype)
    AF = mybir.ActivationFunctionType
    ALU = mybir.AluOpType
    lay, NBIG = _big_layout(M)

    nc = bass.Bass("TRN2", target_bir_lowering=False)
    big = nc.dram_tensor("big", [128, NBIG], f32, kind="ExternalInput")
    xk1 = nc.dram_tensor("xk1", [T, 66], bf16, kind="ExternalInput")
    out = nc.dram_tensor("out", [QPC, D], f32, kind="ExternalOutput")

    with TileCtx(nc) as tc, ExitStack() as st:
        ins = st.enter_context(tc.tile_pool(name="ins", bufs=1))
        feat = st.enter_context(tc.tile_pool(name="feat", bufs=4))
        epool = st.enter_context(tc.tile_pool(name="epool", bufs=1))
        small = st.enter_context(tc.tile_pool(name="small", bufs=4))

        # PSUM: two 4-bank score tiles; freed separately so ctx tiles can
        # start in the first bankset while the second is still exping.
        psc_b_cm = tc.tile_pool(name="pscB", bufs=1, space="PSUM")
        psc_b = psc_b_cm.__enter__()
        psc_a_cm = tc.tile_pool(name="pscA", bufs=1, space="PSUM")
        psc_a = psc_a_cm.__enter__()
        sc_a = psc_a.tile([128, 4, 512], mybir.dt.float32, name="scoresA")
        sc_b = psc_b.tile([128, 4, 512], mybir.dt.float32, name="scoresB")

        def sc_slice(kt):
            return (sc_a if kt < 4 else sc_b)[:, kt % 4, :]

        # ---- PE p-state warm-up + activation table prefetch ----
        scratch = ins.tile([128, 512], f32, name="scratch")
        nc.gpsimd.memset(scratch, 0.0)
        scr_r = scratch.bitcast(mdt)
        dummy = small.tile([128, 1], f32, tag="dummy")
        nc.scalar.activation(out=dummy, in_=scratch[:, 0:1], func=AF.Sin)
        for i in range(N_DUMMY_MM):
            nc.tensor.matmul(sc_a[:, 0, :], scr_r[:, 0:128], scr_r,
                             start=True, stop=True)

        # ---- input DMAs (ordered by when compute needs them) ----
        big_t = ins.tile([128, NBIG], f32)
        s1 = lay["x2q"][1]
        s2 = lay["x2k"][1]
        nc.sync.dma_start(out=big_t[:, 0:s1], in_=big[:, 0:s1])
        nc.sync.dma_start(out=big_t[:, s1:s2], in_=big[:, s1:s2])
        xk1_t = ins.tile([128, KT, 66], bf16)
        nc.sync.dma_start(out=xk1_t,
                          in_=xk1.rearrange("(c p) e -> p c e", p=128))
        nc.sync.dma_start(out=big_t[:, s2:NBIG], in_=big[:, s2:NBIG])

        def bs(name):
            a, b = lay[name]
            return big_t[:, a:b]
        ph_k_t, ph_q_t, wv_t = bs("ph_k"), bs("ph_q"), bs("wv")
        x2q_t, x2k_t = bs("x2q"), bs("x2k")
        xqc_t = bs("xqc").rearrange("p (j d) -> p j d", j=QT)

        # ------------- phase A: features + score matmuls -------------
        W = T + QPC     # virtual concat [k | q] column space

        def emit_m(m, chunks):
            """One frequency's features + matmuls. chunks: ordered (a, b)
            column ranges over the [k | q] concatenated width; the q part
            (T..W) must be a single chunk and come first for fast QF."""
            w32 = float(np.float32(ws[m] / (2 * np.pi)))
            yall = feat.tile([128, W], f32, tag="yall")
            nall = feat.tile([128, W], f32, tag="nall")
            rall = feat.tile([128, W], f32, tag="rall")
            Fall = feat.tile([128, W], mdt, tag="Fall")
            QF = feat.tile([128, QPC], mdt, tag="QF")
            qf_done = False
            covered_k = 0
            mm_done = 0
            y_eng = nc.vector if m == 0 else nc.gpsimd
            for (a, b) in chunks:
                ph = ph_q_t if a >= T else ph_k_t
                src = x2q_t[:, a - T:b - T] if a >= T else x2k_t[:, a:b]
                y_eng.tensor_scalar(
                    out=yall[:, a:b], in0=src, scalar1=w32,
                    scalar2=ph[:, 0:1], op0=ALU.mult, op1=ALU.add)
                nc.vector.tensor_scalar(
                    out=nall[:, a:b], in0=yall[:, a:b],
                    scalar1=float(MAGIC), scalar2=-float(MAGIC),
                    op0=ALU.add, op1=ALU.add)
                nc.vector.tensor_tensor(
                    out=rall[:, a:b], in0=yall[:, a:b],
                    in1=nall[:, a:b], op=ALU.subtract)
                nc.scalar.activation(out=Fall[:, a:b], in_=rall[:, a:b],
                                     func=AF.Sin, scale=TWO_PI)
                if a >= T:
                    nc.scalar.activation(
                        out=QF, in_=Fall[:, T:W], func=AF.Copy,
                        scale=wv_t[:, m:m + 1])
                    qf_done = True
                else:
                    covered_k = max(covered_k, b)
                while qf_done and (mm_done + 1) * 128 <= covered_k:
                    kt = mm_done
                    nc.tensor.matmul(
                        sc_slice(kt), Fall[:, kt * 128:(kt + 1) * 128],
                        QF, start=(m == 0), stop=(m == M - 1))
                    mm_done += 1
            assert mm_done == KT

        warm = [(T, W), (0, 512), (512, T)]
        full = [(T, W), (0, T)]
        for m in range(M):
            emit_m(m, warm if m < 2 else full)

        # ------------- phase B+C: exp (bf16) + ctx + output ----------
        e_a = epool.tile([128, 4, 512], bf16, name="e_a")
        nc.scalar.activation(out=e_a, in_=sc_a, func=AF.Exp)
        e_b = epool.tile([128, 4, 512], bf16, name="e_b")
        nc.scalar.activation(out=e_b, in_=sc_b, func=AF.Exp)

        psc_a_cm.__exit__(None, None, None)
        pctx_cm = tc.tile_pool(name="pctx", bufs=4, space="PSUM")
        pctx = pctx_cm.__enter__()
        out4 = epool.tile([128, QT, D], f32, name="out4")
        cps_l = []
        for j in range(QT):
            cps = pctx.tile([128, 66], mybir.dt.float32, tag="cps")
            cps_l.append(cps)
            for kt in range(4):
                nc.tensor.matmul(
                    cps, e_a[:, kt, j * 128:(j + 1) * 128],
                    xk1_t[:, kt, :], start=(kt == 0), stop=False)
        for j in range(QT):
            cps = cps_l[j]
            for kt in range(4, KT):
                nc.tensor.matmul(
                    cps, e_b[:, kt % 4, j * 128:(j + 1) * 128],
                    xk1_t[:, kt, :], start=False, stop=(kt == KT - 1))
            invs = small.tile([128, 1], f32, tag="invs")
            nc.vector.reciprocal(out=invs, in_=cps[:, 64:65])
            nc.vector.scalar_tensor_tensor(
                out=out4[:, j, :], in0=cps[:, 0:64], scalar=invs[:, 0:1],
                in1=xqc_t[:, j, :], op0=ALU.mult, op1=ALU.add)
            nc.sync.dma_start(
                out=out.rearrange("(j p) d -> p j d", p=128)[:, j, :],
                in_=out4[:, j, :])
        pctx_cm.__exit__(None, None, None)
        psc_b_cm.__exit__(None, None, None)
    return nc


def host_prep(x, scale, gamma, beta, moving_mean, moving_var):
    """Per-core input maps. BN scale A, the linear-term factor
    g = exp(alpha * x.scale) and the softmax denominator (g column) are
    folded into the bf16 key matrix; BN shift goes into the residual."""
    import ml_dtypes
    M = FOURIER_M
    cs = FOURIER_C
    lay, NBIG = _big_layout(M)
    x = np.ascontiguousarray(x, dtype=np.float32)
    A = (np.asarray(gamma, np.float64)
         / np.sqrt(np.asarray(moving_var, np.float64) + BN_EPS)
         ).astype(np.float32)
    Cc = (np.asarray(beta, np.float32) - np.asarray(moving_mean, np.float32)
          * A).astype(np.float32)
    scale = np.asarray(scale, np.float32)

    wvec = np.empty((128, M), np.float32)
    for m in range(M):
        wvec[:64, m] = cs[m] * scale
        wvec[64:, m] = cs[m] * scale
    ph_k = np.zeros((128, 1), np.float32); ph_k[64:] = 0.25   # [sin; cos]
    ph_q = np.zeros((128, 1), np.float32); ph_q[:64] = 0.25   # [cos; sin]

    in_maps = []
    for c in range(NCORES):
        b, h = divmod(c, 2)
        q0 = h * QPC
        xb = x[b]                                    # [T, D]
        xt = np.ascontiguousarray(xb.T)              # [D, T]
        x2k = np.concatenate([xt, xt], 0)            # [128, T]
        g = np.exp(ALPHA * (xb.astype(np.float64) @ scale.astype(np.float64))
                   ).astype(np.float32)              # [T]
        xk1 = np.concatenate(
            [xb * A[None, :] * g[:, None], g[:, None],
             np.zeros((T, 1), np.float32)], 1)       # [T, 66]
        xqc = (xb[q0:q0 + QPC] + Cc).astype(np.float32)
        xqc_pjd = np.transpose(
            xqc.reshape(QT, 128, D), (1, 0, 2)).reshape(128, QT * D)
        bigv = np.empty((128, NBIG), np.float32)
        bigv[:, lay["ph_k"][0]:lay["ph_k"][1]] = ph_k
        bigv[:, lay["ph_q"][0]:lay["ph_q"][1]] = ph_q
        bigv[:, lay["wv"][0]:lay["wv"][1]] = wvec
        bigv[:, lay["x2q"][0]:lay["x2q"][1]] = x2k[:, q0:q0 + QPC]
        bigv[:, lay["x2k"][0]:lay["x2k"][1]] = x2k
        bigv[:, lay["xqc"][0]:lay["xqc"][1]] = xqc_pjd
        in_maps.append({
            "big": bigv,
            "xk1": xk1.astype(ml_dtypes.bfloat16),
        })
    return in_maps


def kernel(x, scale, gamma, beta, moving_mean, moving_var):
    from concourse.bass_utils import run_bass_kernel_spmd
    key = "float32r"
    if key not in _nc_cache:
        _nc_cache[key] = build_nc(mm_dtype=key)
    nc = _nc_cache[key]
    in_maps = host_prep(x, scale, gamma, beta, moving_mean, moving_var)
    res = run_bass_kernel_spmd(nc, in_maps, core_ids=list(range(NCORES)))
    out = np.empty((B, T, D), np.float32)
    for c in range(NCORES):
        b, h = divmod(c, 2)
        q0 = h * QPC
        out[b, q0:q0 + QPC] = res.results[c]["out"]
    return out
